# revision 29
# baseline (speedup 1.0000x reference)
"""Multi-head causal attention (B=2, T=2048, C=1024, H=16) on 8 trn2 cores.

Sharding: tensor-parallel over heads. Each core computes 2 heads' QKV
projections + attention + a partial output projection; the host sums the
8 partial projections and adds the output bias.

v2 design notes:
- PE p-state: the tensor engine only reaches 2.4 GHz after ~3us of
  gapless execution; any stall drops it to 1.2 GHz. The emission order
  therefore interleaves filler matmuls (next batch's projections, prev
  batch's output projection) into the attention loops so PE never waits
  on the scalar-engine exp pipeline.
- V is produced directly in [token, channel] layout by using the x tile
  as the stationary operand (va = x_blk.T @ Wv chunk), avoiding the DMA
  transpose; an appended ones column gives softmax row sums for free.
- Scores/attv/mask use causally-tight column ranges; exp runs 2-wide
  over [128, 2x512] PSUM pairs to amortize fixed activation overhead.
  Stale PSUM columns read by a full-width exp are never consumed by the
  causally-tight attv, so no masking/zeroing of them is needed.
- softmax normalization: row sums are broadcast across partitions with a
  selector matmul and inverted as exp(-ln(s)) on the scalar engine (ln
  and exp share one activation table); the single-partition DVE
  reciprocal is glacial (6x cycle multiplier, no partition parallelism).
"""

import contextlib
import os

import ml_dtypes
import numpy as np

import bass_rust
import concourse.bass as bass
import concourse.mybir as mybir
import concourse.tile as tile
from concourse.bass_utils import run_bass_kernel_spmd

F32 = mybir.dt.float32
F32R = mybir.dt.float32r
BF16 = mybir.dt.bfloat16
FP8 = mybir.dt.float8e4
FP8E = mybir.dt.float8e5
NPBF16 = ml_dtypes.bfloat16
NPFP8 = ml_dtypes.float8_e4m3fn
NPFP8E = ml_dtypes.float8_e5m2

B, T, C, H = 2, 2048, 1024, 16
D = C // H          # 64
NCORES = 8
HL = H // NCORES    # heads per core = 2
TOK = B * T         # 4096
HC = HL * D         # local head channels = 128

NT = T // 512       # 4 token column tiles (512) per batch
KT = C // 128       # 8 contraction tiles for projections
QT = T // 512       # 4 q tiles per batch
JBB = T // 128      # 16 j (key) blocks per batch

_MAXW = 1


def _patched_drain_and_barrier(self, tick_clock, wait_clock):
    """Stock tile tail drain carries one sem-wait per outstanding proc on a
    single TPB_CTRL drain; this walrus build allows only one sync-wait per
    ctrl instruction. Split the waits across no-op carriers."""
    nc = self.nc
    carrier = nc.sync.nop()
    wait_clock.add_sem_waits(
        carrier.ins, bass_rust.ScopedClock({None: tick_clock.global_clock})
    )
    si = carrier.ins.sync_info
    waits = list(si.on_wait) if si and si.on_wait else []
    if len(waits) > _MAXW:
        carrier.ins.sync_info = mybir.SyncInfo(
            on_wait=waits[:_MAXW], on_update=list(si.on_update or [])
        )
        for i in range(_MAXW, len(waits), _MAXW):
            nop = nc.sync.nop()
            nop.ins.sync_info = mybir.SyncInfo(
                on_wait=waits[i : i + _MAXW], on_update=[]
            )
    nc.sync.drain()

    nc.all_engine_barrier()
    popped = nc._tile_sem_poison_stack.pop()
    assert popped is self._sem_poison
    assert self.sems is not None
    nc.clear_and_free_semaphores(list(self.sems.allocated().values()))
    nc.all_engine_barrier()


tile.TileContext._drain_and_barrier = _patched_drain_and_barrier


def _split_waits(nc, maxw=_MAXW):
    """This walrus build accepts at most one sync-wait per instruction.
    Hoist excess waits onto no-op carriers inserted just before the
    instruction on the same engine."""
    for f in nc.m.functions:
        for bb in f.blocks:
            insts = bb.instructions
            if not any(
                i.sync_info and i.sync_info.on_wait and len(i.sync_info.on_wait) > maxw
                for i in insts
            ):
                continue
            new = []
            for inst in insts:
                si = inst.sync_info
                waits = list(si.on_wait) if si and si.on_wait else []
                if len(waits) > maxw:
                    keep = waits[-maxw:]
                    extra = waits[:-maxw]
                    for j in range(0, len(extra), maxw):
                        nop = mybir.InstNoOp(name=nc.get_next_instruction_name())
                        nop.engine = inst.engine
                        nop.sync_info = mybir.SyncInfo(
                            on_wait=extra[j : j + maxw], on_update=[]
                        )
                        nc.register_instruction(nop)
                        new.append(nop)
                    inst.sync_info = mybir.SyncInfo(
                        on_wait=keep, on_update=list(si.on_update or [])
                    )
                new.append(inst)
            bb.instructions = new


def build():
    nc = bass.Bass()
    xT = nc.declare_dram_parameter("xT", [C, TOK], BF16, isOutput=False)
    wq = nc.declare_dram_parameter("wq", [C, HC], BF16, isOutput=False)
    wk = nc.declare_dram_parameter("wk", [C, HC], BF16, isOutput=False)
    wv = nc.declare_dram_parameter("wv", [C, HC], BF16, isOutput=False)
    wo = nc.declare_dram_parameter("wo", [HC, C], BF16, isOutput=False)
    bq = nc.declare_dram_parameter("bq", [HC, 1], F32, isOutput=False)
    bk = nc.declare_dram_parameter("bk", [HC, 1], F32, isOutput=False)
    bv = nc.declare_dram_parameter("bv", [1, HC], BF16, isOutput=False)
    ones1 = nc.declare_dram_parameter("ones1", [1, 128], BF16, isOutput=False)
    masks = nc.declare_dram_parameter("masks", [4, 128, 512], BF16, isOutput=False)
    sel = nc.declare_dram_parameter("sel", [33, 128], F32R, isOutput=False)
    zeros33 = nc.declare_dram_parameter("zeros33", [33, TOK], F32R, isOutput=False)
    out = nc.declare_dram_parameter("out", [TOK, C], BF16, isOutput=True)

    Exp = mybir.ActivationFunctionType.Exp
    Ln = mybir.ActivationFunctionType.Ln
    Copy = mybir.ActivationFunctionType.Copy

    with contextlib.ExitStack() as _st:
        _st.enter_context(
            nc.allow_low_precision(reason="bf16 matmuls with fp32 accumulation")
        )
        tc = _st.enter_context(tile.TileContext(nc))
        with (
            tc.tile_pool(name="consts", bufs=1) as consts,
            tc.tile_pool(name="persist", bufs=1) as persist,
            tc.tile_pool(name="work", bufs=2) as work,
            tc.tile_pool(name="ps_proj", bufs=2, space="PSUM") as ps_proj,
            tc.tile_pool(name="ps_s", bufs=2, space="PSUM") as ps_s,
            tc.tile_pool(name="ps_o", bufs=2, space="PSUM") as ps_o,
        ):
            # ---- constants into SBUF ----
            wq_sb = consts.tile([128, KT, 128], BF16, name="wq_sb")
            wk_sb = consts.tile([128, KT, 128], BF16, name="wk_sb")
            wv_sb = consts.tile([128, KT, 128], BF16, name="wv_sb")
            for w_sb, w_dr in ((wq_sb, wq), (wk_sb, wk), (wv_sb, wv)):
                wr = w_dr.rearrange("(a p) m -> p a m", p=128)
                nc.scalar.dma_start(w_sb[:, 0:2], wr[:, 0:2])
                nc.scalar.dma_start(w_sb[:, 2:KT], wr[:, 2:KT])
            wo_sb = consts.tile([128, C], BF16, name="wo_sb")
            nc.scalar.dma_start(wo_sb, wo[:])
            bq_sb = consts.tile([HC, 1], F32, name="bq_sb")
            bk_sb = consts.tile([HC, 1], F32, name="bk_sb")
            for b_sb, b_dr in ((bq_sb, bq), (bk_sb, bk)):
                nc.scalar.dma_start(b_sb, b_dr[:])
            bv_sb = consts.tile([1, HC], BF16, name="bv_sb")
            nc.scalar.dma_start(bv_sb, bv[:])
            ones1_sb = consts.tile([1, 128], BF16, name="ones1_sb")
            nc.scalar.dma_start(ones1_sb, ones1[:])
            masks_sb = consts.tile([128, 4, 512], BF16, name="masks_sb")
            nc.scalar.dma_start(masks_sb, masks.rearrange("r p f -> p r f"))
            sel_sb = consts.tile([33, 128], F32R, name="sel_sb")
            nc.scalar.dma_start(sel_sb, sel[:])

            # ---- persistent activations ----
            qT = persist.tile([HC, TOK], BF16, name="qT")
            kT = persist.tile([HC, TOK], BF16, name="kT")
            attoT = persist.tile([HC, TOK], BF16, name="attoT")
            sums = persist.tile([33, TOK], F32R, name="sums")
            # rows other than 0/32 are never written but are contracted by
            # the selector matmul (with zero weights); zero them once so
            # stray Inf/NaN garbage cannot poison 0*garbage
            nc.scalar.dma_start(sums, zeros33[:])
            # va: per local head, [key-token partitions, 32 global blocks,
            # 64 v channels + ones column], pitch 80
            va_sb = [
                persist.tile([128, 2 * JBB, 80], BF16, name=f"va{hl}_sb")
                for hl in range(HL)
            ]
            ones_scr = consts.tile([128, 2 * JBB], BF16, name="ones_scr")
            nc.vector.memset(ones_scr, 1.0)
            for hl in range(HL):
                nc.vector.tensor_copy(va_sb[hl][:, :, D], ones_scr)

            # ================= emission units =================

            def phase_a_nt(b, nt):
                """QKV projections for one 512-token tile of batch b."""
                t0 = b * T
                c0 = t0 + nt * 512
                xcols = []
                for kt in range(KT):
                    xcol = work.tile(
                        [128, 512], BF16, tag="xcol", bufs=16, name="xcol"
                    )
                    nc.sync.dma_start(
                        xcol, xT[kt * 128 : (kt + 1) * 128, c0 : c0 + 512]
                    )
                    xcols.append(xcol)
                q_ps = ps_proj.tile([128, 512], F32, tag="proj", name="q_ps")
                for kt in range(KT):
                    nc.tensor.matmul(
                        q_ps, lhsT=wq_sb[:, kt, :], rhs=xcols[kt],
                        start=(kt == 0), stop=(kt == KT - 1),
                    )
                nc.vector.tensor_scalar_add(qT[:, c0 : c0 + 512], q_ps, bq_sb)
                k_ps = ps_proj.tile([128, 512], F32, tag="proj", name="k_ps")
                for kt in range(KT):
                    nc.tensor.matmul(
                        k_ps, lhsT=wk_sb[:, kt, :], rhs=xcols[kt],
                        start=(kt == 0), stop=(kt == KT - 1),
                    )
                nc.vector.tensor_scalar_add(kT[:, c0 : c0 + 512], k_ps, bk_sb)
                # v in [token, channel] layout: x block as stationary
                for blk in range(4):
                    gblk = b * JBB + nt * 4 + blk
                    va_ps = ps_proj.tile([128, 128], F32, tag="proj", name="va_ps")
                    # bias row via rank-1 matmul, then accumulate projections
                    nc.tensor.matmul(
                        va_ps, lhsT=ones1_sb, rhs=bv_sb, start=True, stop=False
                    )
                    for kt in range(KT):
                        nc.tensor.matmul(
                            va_ps,
                            lhsT=xcols[kt][:, blk * 128 : (blk + 1) * 128],
                            rhs=wv_sb[:, kt, :],
                            start=False,
                            stop=(kt == KT - 1),
                        )
                    for hl in range(HL):
                        nc.vector.tensor_copy(
                            va_sb[hl][:, gblk, 0:D],
                            va_ps[:, hl * D : (hl + 1) * D],
                        )

            def attn_group(b, hl, i, pump):
                """Attention for one (batch, local head, 512-query tile)."""
                t0 = b * T
                h0 = hl * D
                q0 = t0 + i * 512
                njb = 4 * (i + 1)
                npair = njb // 2
                o_ps = ps_o.tile([D + 1, 512], F32, tag="ops", name="o_ps")
                pend = []

                def attv(p, cs, e8):
                    for j in range(2):
                        nc.tensor.matmul(
                            o_ps[:, cs:512],
                            lhsT=va_sb[hl][:, b * JBB + 2 * p + j, 0 : D + 1],
                            rhs=e8[:, j, cs:512],
                            start=(p == 0 and j == 0),
                            stop=(p == npair - 1 and j == 1),
                        )

                for p in range(npair):
                    jbs = (2 * p, 2 * p + 1)
                    # pair-level tight start (min over the two blocks);
                    # the mask zeroes block 1's columns below its own start
                    cs = max(0, 128 * (jbs[0] - 4 * i))
                    s_ps = ps_s.tile([128, 2, 512], F32, tag="sps", name="s_ps")
                    for j, jb in enumerate(jbs):
                        nc.tensor.matmul(
                            s_ps[:, j, cs:512],
                            lhsT=kT[
                                h0 : h0 + D, t0 + jb * 128 : t0 + (jb + 1) * 128
                            ],
                            rhs=qT[h0 : h0 + D, q0 + cs : q0 + 512],
                            start=True,
                            stop=True,
                        )
                    e8 = work.tile(
                        [128, 2, 512], BF16, tag="esb", bufs=8, name="e8"
                    )
                    # one exp over both blocks; stale psum columns ahead of
                    # the tight range land in e8 columns the tight attv rhs
                    # never consumes
                    sflat = s_ps.rearrange("p a f -> p (a f)")
                    eflat = e8.rearrange("p a f -> p (a f)")
                    nc.scalar.activation(
                        eflat[:, cs:1024], sflat[:, cs:1024], Exp, scale=0.125
                    )
                    for j, jb in enumerate(jbs):
                        r = jb - 4 * i
                        if r >= 0:
                            nc.vector.tensor_mul(
                                e8[:, j, cs:512],
                                e8[:, j, cs:512],
                                masks_sb[:, r, cs:512],
                            )
                    pend.append((p, cs, e8))
                    # pipeline: run att@v two pairs behind the exp
                    if len(pend) > 2:
                        attv(*pend.pop(0))
                    pump()
                for args in pend:
                    attv(*args)
                nc.vector.tensor_copy(
                    sums[32 * hl : 32 * hl + 1, q0 : q0 + 512], o_ps[D : D + 1, :]
                )
                nc.vector.tensor_copy(
                    attoT[h0 : h0 + D, q0 : q0 + 512], o_ps[0:D, :]
                )

            def norm_tile(b, i):
                """Normalize attoT for one 512-query tile: bcast sums via
                selector matmul, invert as exp(-ln(s)) on the scalar engine."""
                q0 = b * T + i * 512
                rb_ps = ps_proj.tile([128, 512], F32, tag="proj", name="rb_ps")
                nc.tensor.matmul(
                    rb_ps, lhsT=sel_sb, rhs=sums[:, q0 : q0 + 512],
                    start=True, stop=True,
                )
                l_sb = work.tile([128, 512], F32, tag="rsb", bufs=2, name="l_sb")
                nc.scalar.activation(l_sb, rb_ps, Ln)
                r_sb = work.tile([128, 512], F32, tag="rsb", bufs=2, name="r_sb")
                nc.scalar.activation(r_sb, l_sb, Exp, scale=-1.0)
                nc.vector.tensor_mul(
                    attoT[:, q0 : q0 + 512], attoT[:, q0 : q0 + 512], r_sb
                )

            def outproj_tile(b, tt, no2):
                """One [128 tok, 512 C] partial output projection tile."""
                t0 = b * T
                tb = t0 // 128 + tt
                p_ps = ps_proj.tile([128, 512], F32, tag="proj", name="p_ps")
                nc.tensor.matmul(
                    p_ps,
                    lhsT=attoT[:, tb * 128 : (tb + 1) * 128],
                    rhs=wo_sb[:, no2 * 512 : (no2 + 1) * 512],
                    start=True,
                    stop=True,
                )
                o_sb = work.tile([128, 512], BF16, tag="osb", bufs=4, name="o_sb")
                # alternate psum-drain copies between DVE and Act engines
                if (tt * 2 + no2) % 2 == 0:
                    nc.vector.tensor_copy(o_sb, p_ps)
                else:
                    nc.scalar.activation(o_sb, p_ps, Copy)
                nc.sync.dma_start(
                    out[tb * 128 : (tb + 1) * 128, no2 * 512 : (no2 + 1) * 512],
                    o_sb,
                )

            # ================= schedule =================
            # fillers: PE work units (projections for the next batch, output
            # projections of completed q-tiles) interleaved into the
            # attention loops so PE never drains while exp runs on Scalar
            filler = []

            def pump():
                if filler:
                    filler.pop(0)()

            def attn_batch(b):
                for i in range(QT):
                    for hl in range(HL):
                        attn_group(b, hl, i, pump)
                    norm_tile(b, i)
                    # this q-tile's output projection is now unblocked
                    for tt in range(i * 4, (i + 1) * 4):
                        for no2 in range(2):
                            filler.append(
                                lambda b=b, tt=tt, no2=no2: outproj_tile(
                                    b, tt, no2
                                )
                            )

            _s = nc.enter_named_scope("phaseA0", True)
            for nt in range(NT):
                phase_a_nt(0, nt)
            nc.leave_named_scope("phaseA0", _s[0], True)

            _s = nc.enter_named_scope("attn0", True)
            filler = [(lambda nt=nt: phase_a_nt(1, nt)) for nt in range(NT)]
            attn_batch(0)
            nc.leave_named_scope("attn0", _s[0], True)

            _s = nc.enter_named_scope("attn1", True)
            attn_batch(1)
            nc.leave_named_scope("attn1", _s[0], True)

            _s = nc.enter_named_scope("tail", True)
            while filler:
                filler.pop(0)()
            nc.leave_named_scope("tail", _s[0], True)

    _split_waits(nc)
    return nc


def make_in_maps(x, Wq, bq, Wk, bk, Wv, bv, Wo, bo):
    xT = np.ascontiguousarray(x.reshape(TOK, C).T).astype(NPBF16)
    # masks[r, a, c] = 1 if c >= 128r + a  (causal within diagonal blocks)
    a = np.arange(128)[:, None]
    c = np.arange(512)[None, :]
    masks = np.stack(
        [(c >= 128 * rr + a).astype(NPBF16) for rr in range(4)]
    )
    sel = np.zeros((33, 128), np.float32)
    for k in range(HL):
        sel[32 * k, k * D : (k + 1) * D] = 1.0
    in_maps = []
    for core in range(NCORES):
        sl = slice(core * HC, (core + 1) * HC)
        in_maps.append(
            {
                "xT": xT,
                "wq": np.ascontiguousarray(Wq[sl, :].T).astype(NPBF16),
                "wk": np.ascontiguousarray(Wk[sl, :].T).astype(NPBF16),
                "wv": np.ascontiguousarray(Wv[sl, :].T).astype(NPBF16),
                "wo": np.ascontiguousarray(Wo[:, sl].T).astype(NPBF16),
                "bq": np.ascontiguousarray(bq[sl]).reshape(HC, 1),
                "bk": np.ascontiguousarray(bk[sl]).reshape(HC, 1),
                "bv": np.ascontiguousarray(bv[sl]).reshape(1, HC).astype(NPBF16),
                "ones1": np.ones((1, 128), NPBF16),
                "masks": masks,
                "sel": sel,
                "zeros33": np.zeros((33, TOK), np.float32),
            }
        )
    return in_maps


_NC_CACHE = None


def kernel(x, Wq, bq, Wk, bk, Wv, bv, Wo, bo):
    global _NC_CACHE
    x = np.asarray(x, np.float32)
    in_maps = make_in_maps(
        x,
        np.asarray(Wq, np.float32),
        np.asarray(bq, np.float32),
        np.asarray(Wk, np.float32),
        np.asarray(bk, np.float32),
        np.asarray(Wv, np.float32),
        np.asarray(bv, np.float32),
        np.asarray(Wo, np.float32),
        np.asarray(bo, np.float32),
    )
    if _NC_CACHE is None:
        _NC_CACHE = build()
    trace = bool(int(os.environ.get("KERNEL_TRACE", "0")))
    res = run_bass_kernel_spmd(
        _NC_CACHE, in_maps, core_ids=list(range(NCORES)), trace=trace
    )
    if trace:
        kernel.last_results = res
    total = np.zeros((TOK, C), np.float32)
    for core in range(NCORES):
        total += res.results[core]["out"].astype(np.float32)
    total += np.asarray(bo, np.float32)[None, :]
    return total.reshape(B, T, C)
